# revision 11
# baseline (speedup 1.0000x reference)
"""Trainium2 Bass kernel for nn_CRHT_DGC (CTR-GCN style block), 8-core data parallel.

Per core: all BN folded on host; bf16 compute, f32 PSUM.
conv-first pipeline: xd = relu(Wd x); h = [Ws_j xd | W3 xd] (M=128 packed);
h xbar-transposed to ((t4,vp32),(n,tg,c)); graph mix = blockdiag I4(x)PA^T matmul
(K=M=128); CTRGC einsum via per-(n,c) matmuls, 4-way diagonal tile_position;
branch sums accumulate in T-mixed ACC; one xbar back-transpose; residual conv +
identity-inject + fused relu eviction.

Output path: post-relu values are >=0, so the kernel quantizes each half's
[128, n*t*v] staging tile to uint8 with a per-partition scale 254/max (computed
on device, shipped back as f32); the host dequantizes with exactly 1/scale.
This halves the dominant cost — the axon-tunnel download — at ~4e-3 added l2.

Dispatch: the batch is processed in NCHUNK pipelined jit(shard_map(bass_exec))
calls (per-core batch NLC each); the tunnel is full-duplex, so chunk k's
26/NCHUNK MB download overlaps chunk k+1's upload + exec. Donated output
buffers are created ON DEVICE (the stock run_bass_kernel_spmd path re-traces
every call and ships ~52MB of host zeros per call for donation). Falls back to
run_bass_kernel_spmd if the custom path fails.
"""
import os
import numpy as np
import ml_dtypes

import concourse.bass as bass
import concourse.tile as tile
import concourse.masks as masks
from concourse import mybir, bacc
from concourse.bass_utils import run_bass_kernel_spmd

BF16 = mybir.dt.bfloat16
F32 = mybir.dt.float32
U8 = mybir.dt.uint8
bf = ml_dtypes.bfloat16
AF = mybir.ActivationFunctionType
OP = mybir.AluOpType

L, S, V = 3, 3, 25
CIN, COUT, INTER, REL = 64, 256, 64, 8
N, T = 32, 128
EPS = 1e-5
NCORES = 8
NPC = N // NCORES         # 4 samples per core total
NCHUNK = int(os.environ.get('BASS_NCHUNK', '4'))  # pipelined chunks per call
NLC = NPC // NCHUNK       # per-core batch per chunk
VP = 32
TG = T // 4               # 32
QMAX = 254.0              # u8 levels; headroom so max*scale stays < 255

_CACHE = {}

PARBF_LAYOUT = [('wdT', (L, CIN, INTER)), ('wsT', (L, 2, CIN, 128)),
                ('pas', (L, S, VP, VP)), ('w12T', (L, CIN, 40)),
                ('w4T', (L, REL, INTER)), ('wrT', (CIN, COUT))]
PARF32_LAYOUT = [('bd', (L, INTER)), ('b3c', (L, 128)),
                 ('b12', (L, 40)), ('bfin', (2, 128)), ('xscl', (CIN,))]


def _mkoff(layout):
    off, table = 0, {}
    for name, shp in layout:
        sz = 1
        for d in shp:
            sz *= d
        table[name] = (off, shp)
        off += sz
    return table, off


PARBF_OFF, PARBF_SIZE = _mkoff(PARBF_LAYOUT)
PARF32_OFF, PARF32_SIZE = _mkoff(PARF32_LAYOUT)


def _pack(params, xscl):
    pbf = np.concatenate([np.asarray(params[n]).ravel() for n, _ in PARBF_LAYOUT])
    pd = dict(params, xscl=xscl)
    pf = np.concatenate([np.asarray(pd[n], np.float32).ravel()
                         for n, _ in PARF32_LAYOUT])
    return pbf, pf


def _build(nl, half):
    """Half-kernel: produces output channels [half*128, (half+1)*128).
    half 0: graph-mix subsets 0,1. half 1: subset 2 + CTRGC."""
    ntv = nl * T * V
    nc = bacc.Bacc("TRN2", target_bir_lowering=False, debug=False)
    dp = nc.declare_dram_parameter
    x_ext = dp("x", [nl, CIN, T, V], mybir.dt.int8, isOutput=False)
    parbf_ext = dp("parbf", [PARBF_SIZE], BF16, isOutput=False)
    parf32_ext = dp("parf32", [PARF32_SIZE], F32, isOutput=False)
    out_ext = dp("out", [nl, 128, T, V], U8, isOutput=True)
    oscl_ext = dp("oscl", [1, 128], F32, isOutput=True)

    def pview(name):
        table, ext = (PARBF_OFF, parbf_ext) if name in PARBF_OFF else (PARF32_OFF, parf32_ext)
        off, shp = table[name]
        sz = 1
        for d in shp:
            sz *= d
        return ext[off:off + sz]

    subsets = [0, 1] if half == 0 else [2]
    nsub = len(subsets)

    with tile.TileContext(nc) as tc:
        with tc.tile_pool(name="cst", bufs=1) as cst, \
             tc.tile_pool(name="big", bufs=1) as big, \
             tc.tile_pool(name="work", bufs=1) as work, \
             tc.tile_pool(name="ps", bufs=6, space="PSUM") as ps, \
             tc.tile_pool(name="ps2", bufs=2, space="PSUM") as ps2:

            x_sb = big.tile([CIN, nl, T, V], BF16, tag="x")
            xq = big.tile([CIN, nl, T, V], mybir.dt.int8, tag="xd")  # alias xd buf
            nc.sync.dma_start(xq[:], x_ext[:].rearrange("n c t v -> c n t v"))
            xscl_sb = cst.tile([CIN, 1], F32, tag="xscl")
            nc.sync.dma_start(xscl_sb[:], pview('xscl').rearrange(
                "(c o) -> c o", c=CIN, o=1))
            nc.vector.tensor_scalar(
                x_sb[:].rearrange("c n t v -> c (n t v)"),
                xq[:].rearrange("c n t v -> c (n t v)"),
                xscl_sb[:], None, OP.mult)
            wdT = cst.tile([CIN, L, INTER], BF16, tag="wdT")
            nc.sync.dma_start(wdT[:], pview('wdT').rearrange(
                "(l c o) -> c l o", l=L, c=CIN, o=INTER))
            wsT = cst.tile([CIN, L, 128], BF16, tag="wsT")
            nc.sync.dma_start(wsT[:], pview('wsT').rearrange(
                "(l p c m) -> c l p m", l=L, p=2, c=CIN, m=128)[:, :, half, :])
            pab = cst.tile([128, L, nsub, 128], BF16, tag="pab")
            pasv = pview('pas').rearrange("(l s p m) -> l s p m", l=L, s=S, p=VP, m=VP)
            nc.vector.memset(pab[:], 0.0)
            for l_ in range(L):
                for jj, s_ in enumerate(subsets):
                    for t4 in range(4):
                        nc.sync.dma_start(
                            pab[t4 * 32:(t4 + 1) * 32, l_, jj, t4 * 32:(t4 + 1) * 32],
                            pasv[l_, s_, :, :])
            wrT = cst.tile([CIN, 128], BF16, tag="wrT")
            nc.sync.dma_start(wrT[:], pview('wrT').rearrange(
                "(c o) -> c o", c=CIN, o=COUT)[:, half * 128:(half + 1) * 128])
            ident = cst.tile([128, 128], BF16, tag="ident")
            masks.make_identity(nc, ident[:])
            bd_sb = cst.tile([INTER, L], F32, tag="bd")
            nc.sync.dma_start(bd_sb[:], pview('bd').rearrange(
                "(l o) -> o l", l=L, o=INTER))
            bf_sb = cst.tile([128, 1], F32, tag="bf")
            nc.sync.dma_start(bf_sb[:], pview('bfin').rearrange(
                "(h o) -> o h", h=2, o=128)[:, half:half + 1])
            if half == 1:
                b3c_sb = cst.tile([128, L], F32, tag="b3c")
                nc.sync.dma_start(b3c_sb[:], pview('b3c').rearrange(
                    "(l o) -> o l", l=L, o=128))
                w12T = cst.tile([CIN, L, 40], BF16, tag="w12T")
                nc.sync.dma_start(w12T[:], pview('w12T').rearrange(
                    "(l c m) -> c l m", l=L, c=CIN, m=40))
                w4T = cst.tile([REL, L, INTER], BF16, tag="w4T")
                nc.sync.dma_start(w4T[:], pview('w4T').rearrange(
                    "(l r o) -> r l o", l=L, r=REL, o=INTER))
                b12_sb = cst.tile([40, L], F32, tag="b12")
                nc.sync.dma_start(b12_sb[:], pview('b12').rearrange(
                    "(l o) -> o l", l=L, o=40))

            acc = big.tile([128, nl, TG, 128], BF16, tag="acc")
            # no memset: layer-0 writes cover every cell (incl pad rows)
            xd = big.tile([CIN, nl, T, V], BF16, tag="xd")
            h = big.tile([128, nl, T, VP], BF16, tag="h")
            nc.vector.memset(h[:, :, :, V:VP], 0.0)  # pad cols (NaN-safety)
            hT = big.tile([128, nl, TG, 128], BF16, tag="hT")
            h2T = hT
            if half == 1:
                xm = work.tile([CIN, nl, V], BF16, tag="xm")
                x1m = work.tile([REL, nl, V], F32, tag="x1m")
                x2m = work.tile([REL, nl, V], F32, tag="x2m")
                dtile = work.tile([REL, nl, V, VP], BF16, tag="d")
                nc.vector.memset(dtile[:], 0.0)
                mT4 = work.tile([128, nl, V, INTER], BF16, tag="mT4")
                red = work.tile([CIN, 64, V], BF16, tag="red")
            qmx = work.tile([128, 1], F32, tag="qmx")
            qscl = work.tile([128, 1], F32, tag="qscl")
            outq = work.tile([128, nl, T, V], U8, tag="outq")

            x_flat = x_sb[:].rearrange("c n t v -> c (n t v)")
            xd_flat = xd[:].rearrange("c n t v -> c (n t v)")
            nt400 = ntv // 400

            for i in range(L):
                # conv_down: xd = relu(Wd x + bd)
                for k in range(nt400):
                    pt = ps.tile([128, 512], F32, tag="p")
                    nc.tensor.matmul(pt[0:INTER, 0:400], wdT[:, i, :],
                                     x_flat[:, k * 400:(k + 1) * 400],
                                     start=True, stop=True)
                    dst = xd_flat[:, k * 400:(k + 1) * 400]
                    if k % 8 < 5:
                        nc.scalar.activation(dst, pt[0:INTER, 0:400], AF.Relu,
                                             bias=bd_sb[:, i:i + 1])
                    else:
                        nc.vector.tensor_scalar(dst, pt[0:INTER, 0:400],
                                                bd_sb[:, i:i + 1], 0.0, OP.add, OP.max)

                if half == 1:
                    # xm = mean_t xd (gpsimd tree)
                    for n in range(nl):
                        nc.gpsimd.tensor_add(red[:, 0:64, :], xd[:, n, 0:64, :], xd[:, n, 64:128, :])
                        nc.gpsimd.tensor_add(red[:, 0:32, :], red[:, 0:32, :], red[:, 32:64, :])
                        nc.gpsimd.tensor_add(red[:, 0:16, :], red[:, 0:16, :], red[:, 16:32, :])
                        nc.gpsimd.tensor_add(red[:, 0:8, :], red[:, 0:8, :], red[:, 8:16, :])
                        nc.gpsimd.tensor_add(red[:, 0:4, :], red[:, 0:4, :], red[:, 4:8, :])
                        nc.gpsimd.tensor_add(red[:, 0:2, :], red[:, 0:2, :], red[:, 2:4, :])
                        nc.gpsimd.tensor_add(red[:, 0, :], red[:, 0, :], red[:, 1, :])
                        nc.gpsimd.tensor_scalar(xm[:, n, :], red[:, 0, :], 1.0 / T, None, OP.mult)

                    xmf = xm[:].rearrange("c n v -> c (n v)")
                    pt1 = ps2.tile([REL, nl * V], F32, tag="q")
                    nc.tensor.matmul(pt1[:], w12T[:, i, 0:REL], xmf, start=True, stop=True)
                    nc.vector.tensor_scalar(x1m[:].rearrange("r n v -> r (n v)"), pt1[:],
                                            b12_sb[0:REL, i:i + 1], None, OP.add)
                    pt2 = ps2.tile([REL, nl * V], F32, tag="q")
                    nc.tensor.matmul(pt2[:], w12T[:, i, 32:40], xmf, start=True, stop=True)
                    nc.vector.tensor_scalar(x2m[:].rearrange("r n v -> r (n v)"), pt2[:],
                                            b12_sb[32:40, i:i + 1], None, OP.add)

                    nc.vector.tensor_tensor(
                        dtile[:, :, :, 0:V],
                        x1m[:].rearrange("r n (u o) -> r n u o", o=1).broadcast_to([REL, nl, V, V]),
                        x2m[:].rearrange("r n (o v) -> r n o v", o=1).broadcast_to([REL, nl, V, V]),
                        OP.subtract)
                    nc.scalar.activation(dtile[:, :, :, 0:V], dtile[:, :, :, 0:V], AF.Tanh)

                    for n in range(nl):
                        for ug in range(4):
                            nu = min(8, V - ug * 8)
                            pm = ps2.tile([VP, 512], F32, tag="q")
                            for ul in range(nu):
                                u = ug * 8 + ul
                                nc.tensor.matmul(pm[:, ul * INTER:(ul + 1) * INTER],
                                                 dtile[:, n, u, :], w4T[:, i, :],
                                                 start=True, stop=True)
                            nc.vector.tensor_copy(
                                mT4[0:VP, n, ug * 8:ug * 8 + nu, :].rearrange("p u c -> p (u c)"),
                                pm[:, 0:nu * INTER])
                    for k in range(1, 4):
                        nc.scalar.dma_start(mT4[k * 32:(k + 1) * 32, :, :, :], mT4[0:32, :, :, :])

                # h pass for this half's subsets: p = half
                def do_mix(jj, j):
                    # acc columns: local 64*jj block
                    coff = 64 * (j % 2) if j < 2 else 0
                    for n in range(nl):
                        for kb in range(4):
                            pt = ps.tile([128, 512], F32, tag="p")
                            rhs = hT[:, n, kb * 8:(kb + 1) * 8, coff:coff + 64]
                            nc.tensor.matmul(pt[:], pab[:, i, jj, :], rhs, start=True, stop=True)
                            dst = acc[:, n, kb * 8:(kb + 1) * 8, 64 * jj:64 * (jj + 1)]
                            ptv = pt[:].rearrange("p (t c) -> p t c", t=8)
                            if i == 0:
                                if (n * 4 + kb) % 8 < 5:
                                    nc.scalar.activation(dst, ptv, AF.Copy)
                                else:
                                    nc.vector.tensor_copy(dst, ptv)
                            else:
                                nc.vector.tensor_tensor(dst, ptv, dst, OP.add)

                for n in range(nl):
                    for tb in range(8):
                        k = n * 8 + tb
                        pt = ps.tile([128, 512], F32, tag="p")
                        nc.tensor.matmul(
                            pt[:, 0:400], wsT[:, i, :],
                            xd[:, n, tb * 16:(tb + 1) * 16, :].rearrange("c t v -> c (t v)"),
                            start=True, stop=True)
                        dst = h[:, n, tb * 16:(tb + 1) * 16, 0:V]
                        src = pt[:, 0:400].rearrange("p (t v) -> p t v", t=16)
                        if half == 1:
                            if k % 8 < 5:
                                nc.scalar.activation(dst, src, AF.Identity,
                                                     bias=b3c_sb[:, i:i + 1])
                            else:
                                nc.vector.tensor_scalar(dst, src, b3c_sb[:, i:i + 1],
                                                        None, OP.add)
                        else:
                            if k % 8 < 5:
                                nc.scalar.activation(dst, src, AF.Copy)
                            else:
                                nc.vector.tensor_copy(dst, src)
                    for tg in range(TG):
                        nc.sync.dma_start(
                            hT[:, n, tg, :],
                            h[:, n, tg * 4:(tg + 1) * 4, :].rearrange("c t v -> c (t v)"),
                            transpose=True)
                for jj, j in enumerate(subsets):
                    do_mix(jj, j)

                if half == 1:
                    # CTRGC einsum into acc cols 64..127
                    for n in range(nl):
                        for cb in range(4):
                            pe_ = ps.tile([128, 512], F32, tag="p")
                            for cl in range(16):
                                c = cb * 16 + cl
                                for t4 in range(4):
                                    nc.tensor.matmul(
                                        pe_[t4 * 32:t4 * 32 + V, cl * TG:(cl + 1) * TG],
                                        mT4[t4 * 32:t4 * 32 + V, n, :, c],
                                        h2T[t4 * 32:t4 * 32 + V, n, :, 64 + c],
                                        start=True, stop=True,
                                        tile_position=(t4 * 32, t4 * 32))
                            dst = acc[:, n, :, 64 + cb * 16:64 + (cb + 1) * 16] \
                                .rearrange("p t c -> p c t")
                            pev = pe_[:].rearrange("p (c t) -> p c t", c=16)
                            if i == 0:
                                nc.scalar.activation(dst, pev, AF.Copy)
                            else:
                                nc.vector.tensor_tensor(dst, pev, dst, OP.add)

            # final: back-transpose + residual + relu + u8 quantize
            outc = big.tile([128, nl, TG, 4, VP], BF16, tag="hT")
            outstage = big.tile([128, nl, T, V], BF16, tag="h")
            for n in range(nl):
                for tg in range(TG):
                    nc.sync.dma_start(
                        outc[:, n, tg, :, :].rearrange("o a b -> o (a b)"),
                        acc[:, n, tg, :],
                        transpose=True)
            for k in range(nt400):
                n, tb = k // 8, k % 8
                pt = ps.tile([128, 512], F32, tag="p")
                nc.tensor.matmul(
                    pt[:, 0:400], wrT[:],
                    x_sb[:, n, tb * 16:(tb + 1) * 16, :].rearrange("c t v -> c (t v)"),
                    start=True, stop=False)
                nc.tensor.matmul(
                    pt[:, 0:400], ident[:],
                    outc[:, n, tb * 4:(tb + 1) * 4, :, 0:V],
                    start=False, stop=True)
                nc.scalar.activation(
                    outstage[:, n, tb * 16:(tb + 1) * 16, :].rearrange("o t v -> o (t v)"),
                    pt[:, 0:400], AF.Relu, bias=bf_sb[:, 0:1])
            # per-partition u8 quantization: scale = QMAX / max (outstage >= 0)
            ofl = outstage[:].rearrange("o n t v -> o (n t v)")
            nc.vector.tensor_reduce(qmx[:], ofl, mybir.AxisListType.X, OP.max)
            nc.vector.tensor_scalar_max(qmx[:], qmx[:], 1e-20)
            nc.vector.reciprocal(qscl[:], qmx[:])
            nc.vector.tensor_scalar_mul(qscl[:], qscl[:], QMAX)
            nc.vector.tensor_scalar(outq[:].rearrange("o n t v -> o (n t v)"),
                                    ofl, qscl[:], 0.0, OP.mult, OP.add)
            nc.sync.dma_start(
                out_ext[:].rearrange("n o t v -> o n t v"), outq[:])
            nc.sync.dma_start(
                oscl_ext[0:1, :].rearrange("a o -> o a"), qscl[:])
    nc.compile()
    return nc


def _fold(inp):
    g = {k: np.asarray(v, np.float64) for k, v in inp.items()}
    cdinv = g['cdg'] / np.sqrt(g['cdv'] + EPS)
    wdT = (g['cdw'] * cdinv[:, :, None]).transpose(0, 2, 1)
    bd = (g['cdb'] - g['cdm']) * cdinv + g['cdbe']
    finv = g['bng'] / np.sqrt(g['bnv'] + EPS)
    fsh = -g['bnm'] * finv + g['bnb']
    sinv = g['sg'] / np.sqrt(g['sv'] + EPS)
    ws = g['sw'] * sinv[:, :, :, None]
    bs = (g['sb'] - g['sm']) * sinv + g['sbe']
    for j in range(S):
        ws[:, j] *= finv[64 * j:64 * (j + 1)][None, :, None]
        bs[:, j] *= finv[64 * j:64 * (j + 1)][None, :]
    assert np.abs(bs).max() < 1e-7, "nonzero subset bias unsupported"
    wsT = np.zeros((L, 2, CIN, 128))
    wsT[:, 0, :, 0:64] = ws[:, 0].transpose(0, 2, 1)
    wsT[:, 0, :, 64:128] = ws[:, 1].transpose(0, 2, 1)
    wsT[:, 1, :, 0:64] = ws[:, 2].transpose(0, 2, 1)
    wsT[:, 1, :, 64:128] = g['c3w'].transpose(0, 2, 1)
    b3c = np.zeros((L, 128))
    b3c[:, 64:128] = g['c3b']
    w4 = g['c4w'] * finv[192:256][None, :, None]
    assert np.abs(g['c4b'] * finv[192:256]).max() < 1e-7, "nonzero c4 bias unsupported"
    w12T = np.zeros((L, CIN, 40))
    w12T[:, :, 0:REL] = g['c1w'].transpose(0, 2, 1)
    w12T[:, :, 32:40] = g['c2w'].transpose(0, 2, 1)
    b12 = np.zeros((L, 40))
    b12[:, 0:REL] = g['c1b']
    b12[:, 32:40] = g['c2b']
    dinv = g['dg'] / np.sqrt(g['dv'] + EPS)
    wrT = (g['dw'] * dinv[:, None]).T
    br = (g['db'] - g['dm']) * dinv + g['dbe']
    bfin = (fsh + br).reshape(2, 128)
    pas = np.zeros((L, S, VP, VP))
    pas[:, :, 0:V, 0:V] = g['PA'].transpose(0, 1, 3, 2)
    return {
        'wdT': np.ascontiguousarray(wdT).astype(bf), 'bd': bd.astype(np.float32),
        'wsT': wsT.astype(bf), 'b3c': b3c.astype(np.float32),
        'pas': pas.astype(bf), 'w12T': w12T.astype(bf),
        'b12': b12.astype(np.float32),
        'w4T': np.ascontiguousarray(w4.transpose(0, 2, 1)).astype(bf),
        'wrT': np.ascontiguousarray(wrT).astype(bf), 'bfin': bfin.astype(np.float32),
    }


def _setup_runner(nc, nzsets=0):
    """One-time: mirror run_bass_via_pjrt's lowering but cache the jitted
    callable, shardings, and a device-side zeros builder for donation."""
    import jax
    import jax.numpy as jnp
    from jax.sharding import Mesh, PartitionSpec, NamedSharding
    try:
        from jax import shard_map as _sm
        def shard_map(f, mesh, in_specs, out_specs):
            return _sm(f, mesh=mesh, in_specs=in_specs, out_specs=out_specs,
                       check_vma=False)
    except (ImportError, TypeError):
        from jax.experimental.shard_map import shard_map as _sme
        def shard_map(f, mesh, in_specs, out_specs):
            return _sme(f, mesh=mesh, in_specs=in_specs, out_specs=out_specs,
                        check_rep=False)
    from concourse import bass2jax as b2j
    b2j.install_neuronx_cc_hook()

    partition_name = nc.partition_id_tensor.name if nc.partition_id_tensor else None
    in_names, out_names, out_avals, zero_shapes = [], [], [], []
    for alloc in nc.m.functions[0].allocations:
        if not isinstance(alloc, mybir.MemoryLocationSet):
            continue
        name = alloc.memorylocations[0].name
        if alloc.kind == "ExternalInput":
            if name != partition_name:
                in_names.append(name)
        elif alloc.kind == "ExternalOutput":
            shape = tuple(alloc.tensor_shape)
            dtype = mybir.dt.np(alloc.dtype)
            out_names.append(name)
            out_avals.append(jax.core.ShapedArray(shape, dtype))
            zero_shapes.append((shape, dtype))
    n_params = len(in_names)
    n_outs = len(out_avals)
    in_names_full = in_names + out_names
    if partition_name is not None:
        in_names_full.append(partition_name)
    donate = tuple(range(n_params, n_params + n_outs))

    def _body(*args):
        operands = list(args)
        if partition_name is not None:
            operands.append(b2j.partition_id_tensor())
        outs = b2j._bass_exec_p.bind(
            *operands, out_avals=tuple(out_avals),
            in_names=tuple(in_names_full), out_names=tuple(out_names),
            lowering_input_output_aliases=(), sim_require_finite=True,
            sim_require_nnan=True, nc=nc)
        return tuple(outs)

    devices = jax.devices()[:NCORES]
    mesh = Mesh(np.asarray(devices), ("core",))
    sh = NamedSharding(mesh, PartitionSpec("core"))
    in_specs = (PartitionSpec("core"),) * (n_params + n_outs)
    out_specs = (PartitionSpec("core"),) * n_outs
    sharded = jax.jit(
        shard_map(_body, mesh, in_specs, out_specs),
        donate_argnums=donate, keep_unused=True)
    zeros_fn = None
    if nzsets:
        zeros_fn = jax.jit(
            lambda: tuple(jnp.zeros((NCORES * s[0], *s[1:]), d)
                          for _ in range(nzsets) for s, d in zero_shapes),
            out_shardings=tuple(sh for _ in range(nzsets) for _ in zero_shapes))
    return dict(jax=jax, sharded=sharded, zeros_fn=zeros_fn, sh=sh,
                in_names=in_names, out_names=out_names, n_outs=n_outs)


def _dequant_into(dst, q, scl):
    # dst (nl,C,T,V) f32 view; q (nl,C,T,V) u8; scl (C,) f32 device scale
    inv = (1.0 / scl.astype(np.float64)).astype(np.float32).reshape(-1)
    np.multiply(q.astype(np.float32), inv[None, :, None, None], out=dst)


def _run_custom(inputs, x_f32, absmax):
    RA, RB = _CACHE['runners']
    jax, sh = RA['jax'], RA['sh']
    qs = (127.0 / absmax).astype(np.float32)[None, :, None, None]
    xvf = x_f32.reshape(NCORES, NCHUNK, NLC, CIN, T, V)

    def quant_chunk(c):
        return np.round(xvf[:, c].reshape(NCORES * NLC, CIN, T, V) * qs
                        ).astype(np.int8)

    # host work that must precede the uplink queue
    params = _fold(inputs)
    pbf, pf = _pack(params, (absmax / 127.0).astype(np.float32))
    xc0 = quant_chunk(0)
    zs_all = RA['zeros_fn']()
    pdev = {'parbf': jax.device_put(np.tile(pbf, NCORES), sh),
            'parf32': jax.device_put(np.tile(pf, NCORES), sh)}
    no = RA['n_outs']
    x_dev = [None] * NCHUNK
    jobs = []
    zi = 0
    for hi, R in ((0, RA), (1, RB)):
        for c in range(NCHUNK):
            if x_dev[c] is None:
                x_dev[c] = jax.device_put(xc0 if c == 0 else quant_chunk(c), sh)
            ins = [x_dev[c] if n == 'x' else pdev[n] for n in R['in_names']]
            outs = R['sharded'](*ins, *zs_all[zi * no:(zi + 1) * no])
            zi += 1
            od = dict(zip(R['out_names'], outs))
            for s in od['out'].addressable_shards:
                s.data.copy_to_host_async()
            od['oscl'].copy_to_host_async()
            jobs.append((hi, c, od))
    res = np.empty((N, COUT, T, V), np.float32)
    rv = res.reshape(NCORES, NCHUNK, NLC, COUT, T, V)
    from concurrent.futures import ThreadPoolExecutor
    with ThreadPoolExecutor(4) as ex:
        futs = []
        for hi, c, od in jobs:
            scl_np = np.asarray(od['oscl']).reshape(NCORES, 128)
            for k, s in enumerate(od['out'].addressable_shards):
                q = np.asarray(s.data)
                dst = rv[k, c, :, hi * 128:(hi + 1) * 128]
                futs.append(ex.submit(_dequant_into, dst, q, scl_np[k]))
        for f in futs:
            f.result()
    return res


def _run_fallback(inputs, x_f32, absmax):
    ncs = _CACHE['ncs']
    params = _fold(inputs)
    pbf, pf = _pack(params, (absmax / 127.0).astype(np.float32))
    qs = (127.0 / absmax).astype(np.float32)[None, :, None, None]
    xvf = x_f32.reshape(NCORES, NCHUNK, NLC, CIN, T, V)
    res = np.empty((N, COUT, T, V), np.float32)
    rv = res.reshape(NCORES, NCHUNK, NLC, COUT, T, V)
    for hi in range(2):
        for c in range(NCHUNK):
            xc = np.round(xvf[:, c].reshape(NCORES * NLC, CIN, T, V) * qs
                          ).astype(np.int8)
            in_maps = [{'parbf': pbf, 'parf32': pf,
                        'x': np.ascontiguousarray(xc[k * NLC:(k + 1) * NLC])}
                       for k in range(NCORES)]
            rr = run_bass_kernel_spmd(ncs[hi], in_maps,
                                      core_ids=list(range(NCORES))).results
            for k, r in enumerate(rr):
                _dequant_into(rv[k, c, :, hi * 128:(hi + 1) * 128],
                              np.asarray(r['out']),
                              np.asarray(r['oscl']).reshape(128))
    return res


def kernel(**inputs):
    if 'ncs' not in _CACHE:
        _CACHE['ncs'] = (_build(NLC, 0), _build(NLC, 1))
    # int8 x transport: per-channel absmax scaling, dequantized on device
    x = np.asarray(inputs['x'], np.float32)
    absmax = np.maximum(np.abs(x).max(axis=(0, 2, 3)), 1e-30)
    if not os.environ.get('BASS_NO_CUSTOM'):
        try:
            if 'runners' not in _CACHE:
                _CACHE['runners'] = (
                    _setup_runner(_CACHE['ncs'][0], nzsets=2 * NCHUNK),
                    _setup_runner(_CACHE['ncs'][1]))
            return _run_custom(inputs, x, absmax)
        except Exception as e:
            import traceback
            traceback.print_exc()
            print(f"custom runner failed ({e!r}); falling back", flush=True)
            _CACHE.pop('runners', None)
    return _run_fallback(inputs, x, absmax)
